# revision 30
# baseline (speedup 1.0000x reference)
"""MoE FFN (8 experts, top-2, SwiGLU) Trainium2 kernel.

Expert-parallel with quarter-shard load balancing: core e holds expert
e's weights. The router (logits, softmax, top-2 selection AND combine
weights) is computed on the host in float64; tokens are dispatched to
the cores owning their top-2 experts, padded to a balanced main
capacity CAP = round4(3rd-largest expert load) so that at most two
experts overflow. Overflow tokens are offloaded as H-quarter shards:
every core additionally processes up to CAP_A tokens of one (expert,
H-quarter) pair (4 of the 16 h-tiles), so the two overflowing experts'
excess tokens are spread over all 8 cores at 1/4 cost each; the host
scatter-adds both the main y^T and the aux quarter-partials back into
token order. Each core runs the SwiGLU FFN entirely in bf16 (PE rate
matches fp32r while DMA/SBUF halve and LDWEIGHTS gets FWL), scaling by
the per-token combine weight.

Device structure:
  phase A (x-chunk outer):  hT[h, tok] = silu(x@wg)^T * (x@wv)^T
      stationary = wg/wv d-tiles [128, 128], moving = x^T token chunks.
  phase B (d-tile outer):   yT[d, tok] = wo^T @ hT, * w[tok]
      stationary = wo h-tiles [128, 128], moving = hT token chunks; the
      combine weight is applied as a DVE multiply against a
      partition-broadcast copy of w, and y^T stores are one DMA per
      d-tile.

DMA plan: two HWDGE rings drain concurrently. The sync ring carries, in
consumption order: the first x chunk and h-tile-0 weights (split into
halves for the earliest possible first matmul), the rest of the wg/wv
stream (one 512 KB DMA per h-tile), the w broadcast, the remaining x
chunks, wo, and finally the aux-shard weights/tokens — aux MUST ride
the tail of the sync ring: the tile scheduler hoists dependency-free
DMAs to the front of the stream, and only same-engine ring order can
keep the 3 MB aux burst behind the head loads that gate the first real
matmuls. The scalar ring carries only y stores (a sync-ring final store
was observed to execute ~9us late, gating the exec-ending NOTIFY). A
burst of junk matmuls on a memset tile ramps the HAM clock gate and
bridges the DMA ramp until the first loads land.

Self-contained: shapes hardcoded for x[2,2048,1024], 8 experts,
d_expert=2048, top-2; capacity adapts to the routed load at first call
(kernel compiled per distinct CAP and cached).
"""

import math
from contextlib import ExitStack

import ml_dtypes
import numpy as np

import concourse.bass as bass
import concourse.mybir as mybir
import concourse.tile as tile
from concourse import bacc
from concourse.bass_utils import run_bass_kernel_spmd

# ---- problem constants --------------------------------------------------
B, T, D = 2, 2048, 1024
N_TOK = B * T          # 4096 tokens
E = 8                  # experts == cores
H = 2048               # expert hidden dim
TOP_K = 2
P = 128
ND = D // P            # 8  d-tiles (contraction tiles of d_model)
NH = H // P            # 16 h-tiles
NWO = 4                # wo DMA blocks (4 h-tiles each)
NHA = NH // 4          # 4  h-tiles per aux quarter-shard

CAP_LIMIT = 1280       # SBUF budget bound; beyond this, dispatch in rounds
# 48 warm-up matmuls end ~12.5us, exactly when the ramping DMA can first
# sustain the chunk-0 sweep without stalls; starting real matmuls earlier
# (N_WARM=32, deps ready ~10.8us) was measured SLOWER — the stream then
# stalls on the wgv supply and the idle gaps cool the HAM clock gate.
N_WARM = 48

FP = mybir.dt.float32
BF = mybir.dt.bfloat16
AF = mybir.ActivationFunctionType
OP = mybir.AluOpType
BF_NP = ml_dtypes.bfloat16


def _chunks(cap):
    """Token-chunk schedule: a narrow first chunk so phase A can start on
    a small x load, then 512-wide chunks (the PSUM bank limit)."""
    first = min(320, cap)
    out = [(0, first)]
    s = first
    while s < cap:
        out.append((s, min(512, cap - s)))
        s += out[-1][1]
    return out


def _emit(nc, tc, ctx, cap, xt_d, wgv_d, wo_d, w_d, y_d,
          cap_a, xta_d, wgva_d, woa_d, wa_d, ya_d):
    chunks = _chunks(cap)
    const = ctx.enter_context(tc.tile_pool(name="const", bufs=1))
    x_pool = ctx.enter_context(tc.tile_pool(name="x", bufs=1))
    wgv_pool = ctx.enter_context(tc.tile_pool(name="wgv", bufs=1))
    wo_pool = ctx.enter_context(tc.tile_pool(name="wo", bufs=1))
    ht_pool = ctx.enter_context(tc.tile_pool(name="ht", bufs=1))
    act_pool = ctx.enter_context(tc.tile_pool(name="act", bufs=3))
    yst_pool = ctx.enter_context(tc.tile_pool(name="yst", bufs=2))

    # x arrives host-pre-tiled per chunk: xt_d[p, ND*cs + dt*cw + c], so
    # each chunk load is one contiguous 2*ND*cw-byte run per partition.
    xc = []
    for ci, (cs, cw) in enumerate(chunks):
        xtile = x_pool.tile([P, ND, cw], BF, tag=f"x{ci}", name=f"xc{ci}")
        xc.append(xtile)

    # sync ring, in consumption order: the first x chunk and wg/wv of
    # h-tile 0 land first (split into halves so the d0-3 matmuls of the
    # first PSUM group can start on a 0.5 MB footprint), then the rest
    # of the wg/wv stream, the remaining x chunks, then wo.
    # wgv tiles: per-h-tile for hk 0-7 (fine-grained completion gates the
    # chunk-0 sweep), two 4-h-tile blocks for hk 8-15 (consumed late;
    # fewer copies -> fewer queue descriptors to drain in the tail)
    W2 = 2 * ND * P
    cw0 = chunks[0][1]
    wgv_view = {}
    for hk in range(8):
        t = wgv_pool.tile([P, W2], BF, tag=f"wgv{hk}", name=f"wgv{hk}")
        wgv_view[hk] = (t, 0)
    for b in range(2):
        t = wgv_pool.tile([P, 4 * W2], BF, tag=f"wgvb{b}", name=f"wgvb{b}")
        for j in range(4):
            wgv_view[8 + b * 4 + j] = (t, j * W2)

    def wgvs(hk, off, size):
        t, base = wgv_view[hk]
        return t[:, base + off:base + off + size]

    # x0 and wgv0 interleave on the sync ring (moving x0 to the scalar
    # ring was tried and made the deps LAND LATER — the scalar HWDGE
    # queue group spins up worse for head loads)
    x0_ap = xt_d.ap()[:, 0:ND * cw0].rearrange("p (dt c) -> p dt c", dt=ND)
    nc.sync.dma_start(out=xc[0][:, 0:ND // 2, :], in_=x0_ap[:, 0:ND // 2, :])
    nc.sync.dma_start(out=wgvs(0, 0, ND * P), in_=wgv_d.ap()[:, :ND * P])
    nc.sync.dma_start(out=xc[0][:, ND // 2:, :], in_=x0_ap[:, ND // 2:, :])
    nc.sync.dma_start(out=wgvs(0, ND * P, ND * P), in_=wgv_d.ap()[:, ND * P:W2])
    for hk in range(1, 8):
        nc.sync.dma_start(
            out=wgvs(hk, 0, W2), in_=wgv_d.ap()[:, hk * W2:(hk + 1) * W2]
        )
    for b in range(2):
        hk0 = 8 + b * 4
        nc.sync.dma_start(
            out=wgv_view[hk0][0][:],
            in_=wgv_d.ap()[:, hk0 * W2:(hk0 + 4) * W2],
        )

    # w broadcast rides the sync ring behind the head loads — its DRE
    # replicate-write packets would otherwise steal SDMA slots from the
    # critical first x/weight loads; it is only needed at phase B.
    wbc = const.tile([P, cap], FP)
    nc.sync.dma_start(out=wbc[:], in_=w_d.ap().partition_broadcast(P))
    for ci, (cs, cw) in enumerate(chunks[1:], start=1):
        nc.sync.dma_start(
            out=xc[ci][:],
            in_=xt_d.ap()[:, ND * cs:ND * (cs + cw)]
            .rearrange("p (dt c) -> p dt c", dt=ND),
        )
    wo_t = wo_pool.tile([P, NH, D], BF, tag="wo", name="wo")
    nc.sync.dma_start(
        out=wo_t[:], in_=wo_d.ap().rearrange("(j p) c -> p j c", j=NH)
    )

    # aux shard tiles (loads issued after the warmup block: their payload
    # is only consumed after main phase A, and issuing the descriptors
    # first would delay the scalar engine's Silu table preload)
    W2A = NHA * 2 * ND * P
    wgva = wgv_pool.tile([P, W2A], BF, tag="wgva", name="wgva")
    woa = wo_pool.tile([P, NHA, D], BF, tag="woa", name="woa")
    xa = x_pool.tile([P, ND, cap_a], BF, tag="xa", name="xa")
    wbca = const.tile([P, ND * cap_a], FP)
    hta = ht_pool.tile([P, NHA, cap_a], BF, tag="hta", name="hta")

    ht = [
        ht_pool.tile([P, cap], BF, tag=f"h{hk}", name=f"ht{hk}") for hk in range(NH)
    ]

    # ---- PE warm-up: ramp the HAM clock gate while the first loads land
    ones = const.tile([P, P], BF)
    nc.vector.memset(ones[:], 1.0)
    # preload the ACT engine's Silu table during the DMA wait — the first
    # real activation would otherwise pay the ~1.3us table load and stall
    # the first PSUM drain mid-sweep
    sg_warm = act_pool.tile([P, 512], FP, tag="sg", name="sg_warm")
    nc.scalar.activation(sg_warm[:, :1], ones[:, :1], AF.Silu)
    with ExitStack() as wctx:
        ps_w = wctx.enter_context(tc.tile_pool(name="psw", bufs=1, space="PSUM"))
        warm = ps_w.tile([E, P], FP, name="warm", tag="warm")
        for _ in range(N_WARM):
            nc.tensor.matmul(
                warm[:], lhsT=ones[:, :E], rhs=ones[:], start=True, stop=True
            )

    # aux loads ride the SYNC ring behind the wo loads: same-engine DMA
    # program order is preserved through the ring, so the 3 MB aux burst
    # cannot be scheduler-hoisted ahead of the critical head loads (on the
    # scalar ring the scheduler floated it to the very front, stalling the
    # first real matmuls ~5us behind the wgva transfer)
    nc.sync.dma_start(out=wgva[:], in_=wgva_d.ap())
    nc.sync.dma_start(
        out=woa[:], in_=woa_d.ap().rearrange("(j p) c -> p j c", j=NHA)
    )
    nc.sync.dma_start(
        out=xa[:], in_=xta_d.ap().rearrange("p (dt c) -> p dt c", dt=ND)
    )
    nc.sync.dma_start(out=wbca[:], in_=wa_d.ap().partition_broadcast(P))

    # ---- phase A: hT[h, tok] = silu(x@wg)^T * (x@wv)^T ------------------
    with ExitStack() as actx:
        ps_g = actx.enter_context(tc.tile_pool(name="psg", bufs=2, space="PSUM"))
        ps_v = actx.enter_context(tc.tile_pool(name="psv", bufs=2, space="PSUM"))
        # the final chunk sweep uses its own two banks so phase B's PSUM
        # pool reuses banks freed a whole sweep earlier, not A's last drains
        ps_g2 = actx.enter_context(tc.tile_pool(name="psg2", bufs=1, space="PSUM"))
        ps_v2 = actx.enter_context(tc.tile_pool(name="psv2", bufs=1, space="PSUM"))
        for ci, (cs, cw) in enumerate(chunks):
            last_ci = ci == len(chunks) - 1
            for hk in range(NH):
                pg = (ps_g2 if last_ci else ps_g).tile(
                    [P, 512], FP, tag="pg", name=f"pg{ci}_{hk}"
                )
                pv = (ps_v2 if last_ci else ps_v).tile(
                    [P, 512], FP, tag="pv", name=f"pv{ci}_{hk}"
                )
                for d in range(ND):
                    nc.tensor.matmul(
                        pg[:, :cw],
                        lhsT=wgvs(hk, d * P, P),
                        rhs=xc[ci][:, d, :],
                        start=(d == 0),
                        stop=(d == ND - 1),
                    )
                for d in range(ND):
                    nc.tensor.matmul(
                        pv[:, :cw],
                        lhsT=wgvs(hk, ND * P + d * P, P),
                        rhs=xc[ci][:, d, :],
                        start=(d == 0),
                        stop=(d == ND - 1),
                    )
                sg = act_pool.tile([P, 512], FP, tag="sg")
                nc.scalar.activation(sg[:, :cw], pg[:, :cw], AF.Silu)
                nc.vector.tensor_tensor(
                    ht[hk][:, cs:cs + cw], pv[:, :cw], sg[:, :cw], op=OP.mult
                )


    # ---- aux phase A: the quarter-shard's hT over NHA h-tiles -----------
    with ExitStack() as axctx:
        ps_a = axctx.enter_context(tc.tile_pool(name="psa", bufs=2, space="PSUM"))
        for k in range(NHA):
            pga = ps_a.tile([P, cap_a], FP, tag="pga", name=f"pga{k}")
            pva = ps_a.tile([P, cap_a], FP, tag="pva", name=f"pva{k}")
            for d in range(ND):
                nc.tensor.matmul(
                    pga[:],
                    lhsT=wgva[:, (2 * k) * ND * P + d * P:(2 * k) * ND * P + (d + 1) * P],
                    rhs=xa[:, d, :],
                    start=(d == 0),
                    stop=(d == ND - 1),
                )
            for d in range(ND):
                nc.tensor.matmul(
                    pva[:],
                    lhsT=wgva[:, (2 * k + 1) * ND * P + d * P:(2 * k + 1) * ND * P + (d + 1) * P],
                    rhs=xa[:, d, :],
                    start=(d == 0),
                    stop=(d == ND - 1),
                )
            sga = act_pool.tile([P, 512], FP, tag="sg")
            nc.scalar.activation(sga[:, :cap_a], pga[:], AF.Silu)
            nc.vector.tensor_tensor(hta[:, k, :], pva[:], sga[:, :cap_a], op=OP.mult)

    # ---- phase B: yT[d, tok] = (wo^T @ hT) * w --------------------------
    with ExitStack() as bctx:
        ps_y = bctx.enter_context(tc.tile_pool(name="psy", bufs=2, space="PSUM"))
        ps_ya = bctx.enter_context(tc.tile_pool(name="psya", bufs=1, space="PSUM"))
        # aux phase B first: all ND d-tiles packed column-wise into PSUM
        # groups, one DVE mult + one early store per group
        gsz = max(1, 512 // cap_a)          # d-tiles per PSUM tile
        ysa = yst_pool.tile([P, ND * cap_a], BF, tag="ya", name="ysa")
        for g0 in range(0, ND, gsz):
            gn = min(gsz, ND - g0)
            pya = ps_ya.tile([P, 512], FP, tag="auxy", name=f"pya{g0}")
            for j in range(gn):
                for k in range(NHA):
                    nc.tensor.matmul(
                        pya[:, j * cap_a:(j + 1) * cap_a],
                        lhsT=woa[:, k, (g0 + j) * P:(g0 + j + 1) * P],
                        rhs=hta[:, k, :],
                        start=(k == 0),
                        stop=(k == NHA - 1),
                    )
            nc.vector.tensor_tensor(
                ysa[:, g0 * cap_a:(g0 + gn) * cap_a],
                pya[:, :gn * cap_a],
                wbca[:, g0 * cap_a:(g0 + gn) * cap_a],
                op=OP.mult,
            )
        nc.sync.dma_start(out=ya_d.ap(), in_=ysa[:])
        for dt in range(ND):
            ysb = yst_pool.tile([P, cap], BF, tag="y", name=f"y{dt}")
            if dt < ND - 1:
                pys = [
                    ps_y.tile([P, 512], FP, tag=f"c{ci}", name=f"py{dt}c{ci}")
                    for ci in range(len(chunks))
                ]
                # interleave chunks per hk: one wo weight-load feeds all
                # three chunk matmuls
                for hk in range(NH):
                    for ci, (cs, cw) in enumerate(chunks):
                        nc.tensor.matmul(
                            pys[ci][:, :cw],
                            lhsT=wo_t[:, hk, dt * P:(dt + 1) * P],
                            rhs=ht[hk][:, cs:cs + cw],
                            start=(hk == 0),
                            stop=(hk == NH - 1),
                        )
                for ci, (cs, cw) in enumerate(chunks):
                    nc.vector.tensor_tensor(
                        ysb[:, cs:cs + cw], pys[ci][:, :cw], wbc[:, cs:cs + cw],
                        op=OP.mult,
                    )
                nc.sync.dma_start(
                    out=y_d.ap()[dt * P:(dt + 1) * P, :], in_=ysb[:]
                )
            else:
                # final d-tile: chunk-outer so earlier chunks scale + store
                # while later chunks are still accumulating, and the last
                # chunk is sub-split so the kernel's tail is one 68-column
                # store
                pieces = list(chunks)
                lcs, lcw = pieces[-1]
                if lcw > 96:
                    pieces[-1] = (lcs, lcw - 68)
                    pieces.append((lcs + lcw - 68, 68))
                pyp = [
                    ps_y.tile([P, 512], FP, tag=f"c{ci % len(chunks)}",
                              name=f"pyz{ci}")
                    for ci in range(len(pieces))
                ]
                for ci, (cs, cw) in enumerate(pieces):
                    for hk in range(NH):
                        nc.tensor.matmul(
                            pyp[ci][:, :cw],
                            lhsT=wo_t[:, hk, dt * P:(dt + 1) * P],
                            rhs=ht[hk][:, cs:cs + cw],
                            start=(hk == 0),
                            stop=(hk == NH - 1),
                        )
                    nc.vector.tensor_tensor(
                        ysb[:, cs:cs + cw], pyp[ci][:, :cw], wbc[:, cs:cs + cw],
                        op=OP.mult,
                    )
                    # scalar ring only: sync-ring stores have been observed
                    # to execute ~9us after their doorbell (late-queue
                    # pathology), gating the exec-ending NOTIFY
                    nc.sync.dma_start(
                        out=y_d.ap()[dt * P:(dt + 1) * P, cs:cs + cw],
                        in_=ysb[:, cs:cs + cw],
                    )


def _build(cap, cap_a):
    nc = bacc.Bacc("TRN2", target_bir_lowering=False, debug=False)
    xt_d = nc.dram_tensor("xt", [P, ND * cap], BF, kind="ExternalInput")
    wgv_d = nc.dram_tensor("wgv", [P, NH * 2 * ND * P], BF, kind="ExternalInput")
    wo_d = nc.dram_tensor("wo", [H, D], BF, kind="ExternalInput")
    w_d = nc.dram_tensor("w", [1, cap], FP, kind="ExternalInput")
    y_d = nc.dram_tensor("y", [D, cap], BF, kind="ExternalOutput")
    xta_d = nc.dram_tensor("xta", [P, ND * cap_a], BF, kind="ExternalInput")
    wgva_d = nc.dram_tensor("wgva", [P, NHA * 2 * ND * P], BF, kind="ExternalInput")
    woa_d = nc.dram_tensor("woa", [NHA * P, D], BF, kind="ExternalInput")
    wa_d = nc.dram_tensor("wa", [1, ND * cap_a], FP, kind="ExternalInput")
    ya_d = nc.dram_tensor("ya", [P, ND * cap_a], BF, kind="ExternalOutput")
    with tile.TileContext(nc) as tc:
        with ExitStack() as ctx:
            _emit(nc, tc, ctx, cap, xt_d, wgv_d, wo_d, w_d, y_d,
                  cap_a, xta_d, wgva_d, woa_d, wa_d, ya_d)
    nc.compile()
    return nc


_NCS = {}


def _get_nc(cap, cap_a):
    if (cap, cap_a) not in _NCS:
        _NCS[(cap, cap_a)] = _build(cap, cap_a)
    return _NCS[(cap, cap_a)]


def _route(xf, gate_w, expert_bias):
    """Host router in float64: top-2 selection + normalized combine weights."""
    logits = xf.astype(np.float64) @ gate_w.astype(np.float64) + expert_bias.astype(
        np.float64
    )
    m = logits.max(axis=-1, keepdims=True)
    p = np.exp(logits - m)
    p /= p.sum(axis=-1, keepdims=True)
    # ties -> lower index first, matching jax.lax.top_k
    order = np.argsort(-p, axis=-1, kind="stable")[:, :TOP_K]
    rw = np.take_along_axis(p, order, axis=-1)
    rw = rw / (rw.sum(axis=-1, keepdims=True) + 1e-8)
    return order, rw


def _tile_wgv(wg, wv):
    """Two [D, H] fp32 -> [128, NH*2*ND*128] bf16, [p, hk, {g,v}, dt, h]."""
    def t(w):
        return w.astype(BF_NP).reshape(ND, P, NH, P).transpose(1, 2, 0, 3)

    return np.ascontiguousarray(
        np.stack([t(wg), t(wv)], axis=2).reshape(P, NH * 2 * ND * P)
    )


def kernel(x, gate_w, expert_bias, w_gate, w_value, w_out, _trace=False):
    x = np.asarray(x, dtype=np.float32)
    gate_w = np.asarray(gate_w, dtype=np.float32)
    expert_bias = np.asarray(expert_bias, dtype=np.float32)
    w_gate = np.asarray(w_gate, dtype=np.float32)
    w_value = np.asarray(w_value, dtype=np.float32)
    w_out = np.asarray(w_out, dtype=np.float32)

    xf = np.ascontiguousarray(x.reshape(N_TOK, D))
    order, rw = _route(xf, gate_w, expert_bias)
    idx = [np.flatnonzero((order == e).any(axis=-1)) for e in range(E)]
    loads = [len(i) for i in idx]
    max_load = max(1, max(loads))

    def r4(v):
        return max(4, -4 * (-v // 4))

    # main capacity: smallest (mult-of-4) cap with at most two overflow
    # experts; their overflow tokens are offloaded as H-quarter shards
    # spread over the 8 cores (4 shards per overflow expert).
    if max_load > CAP_LIMIT:
        cap, cap_a = CAP_LIMIT, 4      # multi-round fallback, no aux
    else:
        cap = r4(sorted(loads, reverse=True)[2])
        cap_a = r4(max([loads[e] - cap for e in range(E)] + [4]))
        while cap_a > 512:             # extreme imbalance: grow main cap
            cap = r4(cap + 128)
            cap_a = r4(max([loads[e] - cap for e in range(E)] + [4]))
    n_rounds = max(1, math.ceil(max_load / cap)) if max_load > CAP_LIMIT else 1
    ov = [e for e in range(E) if loads[e] > cap]     # <= 2 experts
    # shard s -> (expert, quarter); cores 0-3 serve ov[0], 4-7 serve ov[1]
    shards = [(ov[s // 4], s % 4) if s // 4 < len(ov) else None
              for s in range(E)]

    nc = _get_nc(cap, cap_a)
    wgv_t = [_tile_wgv(w_gate[e], w_value[e]) for e in range(E)]
    wo_t = [np.ascontiguousarray(w_out[e].astype(BF_NP)) for e in range(E)]
    # per-token combine weight of each token for expert e
    w_of = [
        np.where(
            order[:, 0] == e,
            rw[:, 0],
            np.where(order[:, 1] == e, rw[:, 1], 0.0),
        ).astype(np.float32)
        for e in range(E)
    ]

    def pack_x(ids, n, chunked):
        ids_p = np.zeros(n, dtype=np.int64)
        ids_p[: len(ids)] = ids
        xT = xf[ids_p].T.astype(BF_NP).reshape(ND, P, n)
        xT[:, :, len(ids):] = 0
        pieces = _chunks(n) if chunked else [(0, n)]
        return np.concatenate(
            [
                np.ascontiguousarray(
                    xT[:, :, cs:cs + cw].transpose(1, 0, 2)
                ).reshape(P, ND * cw)
                for cs, cw in pieces
            ],
            axis=1,
        )

    W2 = 2 * ND * P
    out = np.zeros((N_TOK, D), dtype=np.float32)
    last = None
    for r in range(n_rounds):
        in_maps = []
        for e in range(E):
            ids = idx[e][r * cap:(r + 1) * cap]
            w_pad = np.zeros((1, cap), dtype=np.float32)
            w_pad[0, : len(ids)] = w_of[e][ids]
            sh = shards[e] if n_rounds == 1 else None
            if sh is not None:
                ea, q = sh
                ids_a = idx[ea][cap:cap + cap_a]
                wgva = np.ascontiguousarray(
                    wgv_t[ea][:, q * NHA * W2:(q + 1) * NHA * W2])
                woa = np.ascontiguousarray(
                    wo_t[ea][q * NHA * P:(q + 1) * NHA * P, :])
                wa = np.zeros((1, cap_a), dtype=np.float32)
                wa[0, : len(ids_a)] = w_of[ea][ids_a]
            else:
                ids_a = np.zeros(0, dtype=np.int64)
                wgva = np.zeros((P, NHA * W2), dtype=BF_NP)
                woa = np.zeros((NHA * P, D), dtype=BF_NP)
                wa = np.zeros((1, cap_a), dtype=np.float32)
            in_maps.append({
                "xt": pack_x(ids, cap, True),
                "wgv": wgv_t[e],
                "wo": wo_t[e],
                "w": w_pad,
                "xta": pack_x(ids_a, cap_a, False),
                "wgva": wgva,
                "woa": woa,
                "wa": np.ascontiguousarray(np.tile(wa, ND)),
            })
        res = run_bass_kernel_spmd(
            nc, in_maps, core_ids=list(range(E)),
            trace=bool(_trace), trace_cores=list(range(E)) if _trace else None,
        )
        last = res
        for e in range(E):
            ids = idx[e][r * cap:(r + 1) * cap]
            if len(ids):
                out[ids] += res.results[e]["y"][:, : len(ids)].T.astype(np.float32)
            sh = shards[e] if n_rounds == 1 else None
            if sh is not None:
                ea, _ = sh
                ids_a = idx[ea][cap:cap + cap_a]
                if len(ids_a):
                    ya = res.results[e]["ya"].reshape(P, ND, cap_a)
                    contrib = ya.transpose(2, 1, 0).reshape(cap_a, D)
                    out[ids_a] += contrib[: len(ids_a)].astype(np.float32)
    if _trace:
        kernel.last_results = last
    return out.reshape(B, T, D)



# revision 31
# speedup vs baseline: 1.0134x; 1.0134x over previous
"""MoE FFN (8 experts, top-2, SwiGLU) Trainium2 kernel.

Expert-parallel with quarter-shard load balancing: core e holds expert
e's weights. The router (logits, softmax, top-2 selection AND combine
weights) is computed on the host in float64; tokens are dispatched to
the cores owning their top-2 experts, padded to a balanced main
capacity CAP = round4(3rd-largest expert load) so that at most two
experts overflow. Overflow tokens are offloaded as H-quarter shards:
every core additionally processes up to CAP_A tokens of one (expert,
H-quarter) pair (4 of the 16 h-tiles), so the two overflowing experts'
excess tokens are spread over all 8 cores at 1/4 cost each; the host
scatter-adds both the main y^T and the aux quarter-partials back into
token order. Each core runs the SwiGLU FFN entirely in bf16 (PE rate
matches fp32r while DMA/SBUF halve and LDWEIGHTS gets FWL), scaling by
the per-token combine weight.

Device structure:
  phase A (x-chunk outer):  hT[h, tok] = silu(x@wg)^T * (x@wv)^T
      stationary = wg/wv d-tiles [128, 128], moving = x^T token chunks.
  phase B (d-tile outer):   yT[d, tok] = wo^T @ hT, * w[tok]
      stationary = wo h-tiles [128, 128], moving = hT token chunks; the
      combine weight is applied as a DVE multiply against a
      partition-broadcast copy of w, and y^T stores are one DMA per
      d-tile.

DMA plan: two HWDGE rings drain concurrently. The sync ring carries, in
consumption order: the first x chunk and h-tile-0 weights (split into
halves for the earliest possible first matmul), the rest of the wg/wv
stream (one 512 KB DMA per h-tile), the w broadcast, the remaining x
chunks, wo, and finally the aux-shard weights/tokens — aux MUST ride
the tail of the sync ring: the tile scheduler hoists dependency-free
DMAs to the front of the stream, and only same-engine ring order can
keep the 3 MB aux burst behind the head loads that gate the first real
matmuls. The scalar ring carries only y stores (a sync-ring final store
was observed to execute ~9us late, gating the exec-ending NOTIFY). A
burst of junk matmuls on a memset tile ramps the HAM clock gate and
bridges the DMA ramp until the first loads land.

Self-contained: shapes hardcoded for x[2,2048,1024], 8 experts,
d_expert=2048, top-2; capacity adapts to the routed load at first call
(kernel compiled per distinct CAP and cached).
"""

import math
from contextlib import ExitStack

import ml_dtypes
import numpy as np

import concourse.bass as bass
import concourse.mybir as mybir
import concourse.tile as tile
from concourse import bacc
from concourse.bass_utils import run_bass_kernel_spmd

# ---- problem constants --------------------------------------------------
B, T, D = 2, 2048, 1024
N_TOK = B * T          # 4096 tokens
E = 8                  # experts == cores
H = 2048               # expert hidden dim
TOP_K = 2
P = 128
ND = D // P            # 8  d-tiles (contraction tiles of d_model)
NH = H // P            # 16 h-tiles
NWO = 4                # wo DMA blocks (4 h-tiles each)
NHA = NH // 4          # 4  h-tiles per aux quarter-shard

CAP_LIMIT = 1280       # SBUF budget bound; beyond this, dispatch in rounds
# 48 warm-up matmuls end ~12.5us, exactly when the ramping DMA can first
# sustain the chunk-0 sweep without stalls; starting real matmuls earlier
# (N_WARM=32, deps ready ~10.8us) was measured SLOWER — the stream then
# stalls on the wgv supply and the idle gaps cool the HAM clock gate.
N_WARM = 48

FP = mybir.dt.float32
BF = mybir.dt.bfloat16
AF = mybir.ActivationFunctionType
OP = mybir.AluOpType
BF_NP = ml_dtypes.bfloat16


def _chunks(cap):
    """Token-chunk schedule: a narrow first chunk so phase A can start on
    a small x load, then 512-wide chunks (the PSUM bank limit)."""
    first = min(320, cap)
    out = [(0, first)]
    s = first
    while s < cap:
        out.append((s, min(512, cap - s)))
        s += out[-1][1]
    return out


def _emit(nc, tc, ctx, cap, xt_d, wgv_d, wo_d, w_d, y_d,
          cap_a, xta_d, wgva_d, woa_d, wa_d, ya_d):
    chunks = _chunks(cap)
    const = ctx.enter_context(tc.tile_pool(name="const", bufs=1))
    x_pool = ctx.enter_context(tc.tile_pool(name="x", bufs=1))
    wgv_pool = ctx.enter_context(tc.tile_pool(name="wgv", bufs=1))
    wo_pool = ctx.enter_context(tc.tile_pool(name="wo", bufs=1))
    ht_pool = ctx.enter_context(tc.tile_pool(name="ht", bufs=1))
    act_pool = ctx.enter_context(tc.tile_pool(name="act", bufs=3))
    yst_pool = ctx.enter_context(tc.tile_pool(name="yst", bufs=2))

    # x arrives host-pre-tiled per chunk: xt_d[p, ND*cs + dt*cw + c], so
    # each chunk load is one contiguous 2*ND*cw-byte run per partition.
    xc = []
    for ci, (cs, cw) in enumerate(chunks):
        xtile = x_pool.tile([P, ND, cw], BF, tag=f"x{ci}", name=f"xc{ci}")
        xc.append(xtile)

    # sync ring, in consumption order: the first x chunk and wg/wv of
    # h-tile 0 land first (split into halves so the d0-3 matmuls of the
    # first PSUM group can start on a 0.5 MB footprint), then the rest
    # of the wg/wv stream, the remaining x chunks, then wo.
    # wgv tiles: per-h-tile for hk 0-7 (fine-grained completion gates the
    # chunk-0 sweep), two 4-h-tile blocks for hk 8-15 (consumed late;
    # fewer copies -> fewer queue descriptors to drain in the tail)
    W2 = 2 * ND * P
    cw0 = chunks[0][1]
    wgv_view = {}
    for hk in range(8):
        t = wgv_pool.tile([P, W2], BF, tag=f"wgv{hk}", name=f"wgv{hk}")
        wgv_view[hk] = (t, 0)
    for b in range(2):
        t = wgv_pool.tile([P, 4 * W2], BF, tag=f"wgvb{b}", name=f"wgvb{b}")
        for j in range(4):
            wgv_view[8 + b * 4 + j] = (t, j * W2)

    def wgvs(hk, off, size):
        t, base = wgv_view[hk]
        return t[:, base + off:base + off + size]

    # x0 and wgv0 interleave on the sync ring (moving x0 to the scalar
    # ring was tried and made the deps LAND LATER — the scalar HWDGE
    # queue group spins up worse for head loads)
    x0_ap = xt_d.ap()[:, 0:ND * cw0].rearrange("p (dt c) -> p dt c", dt=ND)
    nc.sync.dma_start(out=xc[0][:, 0:ND // 2, :], in_=x0_ap[:, 0:ND // 2, :])
    nc.sync.dma_start(out=wgvs(0, 0, ND * P), in_=wgv_d.ap()[:, :ND * P])
    nc.sync.dma_start(out=xc[0][:, ND // 2:, :], in_=x0_ap[:, ND // 2:, :])
    nc.sync.dma_start(out=wgvs(0, ND * P, ND * P), in_=wgv_d.ap()[:, ND * P:W2])
    for hk in range(1, 8):
        nc.sync.dma_start(
            out=wgvs(hk, 0, W2), in_=wgv_d.ap()[:, hk * W2:(hk + 1) * W2]
        )
    for b in range(2):
        hk0 = 8 + b * 4
        nc.sync.dma_start(
            out=wgv_view[hk0][0][:],
            in_=wgv_d.ap()[:, hk0 * W2:(hk0 + 4) * W2],
        )

    # w broadcast rides the sync ring behind the head loads — its DRE
    # replicate-write packets would otherwise steal SDMA slots from the
    # critical first x/weight loads; it is only needed at phase B.
    wbc = const.tile([P, cap], FP)
    nc.sync.dma_start(out=wbc[:], in_=w_d.ap().partition_broadcast(P))
    for ci, (cs, cw) in enumerate(chunks[1:], start=1):
        nc.sync.dma_start(
            out=xc[ci][:],
            in_=xt_d.ap()[:, ND * cs:ND * (cs + cw)]
            .rearrange("p (dt c) -> p dt c", dt=ND),
        )
    wo_t = wo_pool.tile([P, NH, D], BF, tag="wo", name="wo")
    nc.sync.dma_start(
        out=wo_t[:], in_=wo_d.ap().rearrange("(j p) c -> p j c", j=NH)
    )

    # aux shard tiles (loads issued after the warmup block: their payload
    # is only consumed after main phase A, and issuing the descriptors
    # first would delay the scalar engine's Silu table preload)
    W2A = NHA * 2 * ND * P
    wgva = wgv_pool.tile([P, W2A], BF, tag="wgva", name="wgva")
    woa = wo_pool.tile([P, NHA, D], BF, tag="woa", name="woa")
    xa = x_pool.tile([P, ND, cap_a], BF, tag="xa", name="xa")
    wbca = const.tile([P, ND * cap_a], FP)
    hta = ht_pool.tile([P, NHA, cap_a], BF, tag="hta", name="hta")

    ht = [
        ht_pool.tile([P, cap], BF, tag=f"h{hk}", name=f"ht{hk}") for hk in range(NH)
    ]

    # ---- PE warm-up: ramp the HAM clock gate while the first loads land
    ones = const.tile([P, P], BF)
    nc.vector.memset(ones[:], 1.0)
    # preload the ACT engine's Silu table during the DMA wait — the first
    # real activation would otherwise pay the ~1.3us table load and stall
    # the first PSUM drain mid-sweep
    sg_warm = act_pool.tile([P, 512], FP, tag="sg", name="sg_warm")
    nc.scalar.activation(sg_warm[:, :1], ones[:, :1], AF.Silu)
    with ExitStack() as wctx:
        ps_w = wctx.enter_context(tc.tile_pool(name="psw", bufs=1, space="PSUM"))
        warm = ps_w.tile([E, P], FP, name="warm", tag="warm")
        for _ in range(N_WARM):
            nc.tensor.matmul(
                warm[:], lhsT=ones[:, :E], rhs=ones[:], start=True, stop=True
            )

    # aux loads ride the SYNC ring behind the wo loads: same-engine DMA
    # program order is preserved through the ring, so the 3 MB aux burst
    # cannot be scheduler-hoisted ahead of the critical head loads (on the
    # scalar ring the scheduler floated it to the very front, stalling the
    # first real matmuls ~5us behind the wgva transfer)
    nc.sync.dma_start(out=wgva[:], in_=wgva_d.ap())
    nc.sync.dma_start(
        out=woa[:], in_=woa_d.ap().rearrange("(j p) c -> p j c", j=NHA)
    )
    nc.sync.dma_start(
        out=xa[:], in_=xta_d.ap().rearrange("p (dt c) -> p dt c", dt=ND)
    )
    nc.sync.dma_start(out=wbca[:], in_=wa_d.ap().partition_broadcast(P))

    # ---- phase A: hT[h, tok] = silu(x@wg)^T * (x@wv)^T ------------------
    with ExitStack() as actx:
        ps_g = actx.enter_context(tc.tile_pool(name="psg", bufs=2, space="PSUM"))
        ps_v = actx.enter_context(tc.tile_pool(name="psv", bufs=2, space="PSUM"))
        # the final chunk sweep uses its own two banks so phase B's PSUM
        # pool reuses banks freed a whole sweep earlier, not A's last drains
        ps_g2 = actx.enter_context(tc.tile_pool(name="psg2", bufs=1, space="PSUM"))
        ps_v2 = actx.enter_context(tc.tile_pool(name="psv2", bufs=1, space="PSUM"))
        for ci, (cs, cw) in enumerate(chunks):
            last_ci = ci == len(chunks) - 1
            for hk in range(NH):
                pg = (ps_g2 if last_ci else ps_g).tile(
                    [P, 512], FP, tag="pg", name=f"pg{ci}_{hk}"
                )
                pv = (ps_v2 if last_ci else ps_v).tile(
                    [P, 512], FP, tag="pv", name=f"pv{ci}_{hk}"
                )
                for d in range(ND):
                    nc.tensor.matmul(
                        pg[:, :cw],
                        lhsT=wgvs(hk, d * P, P),
                        rhs=xc[ci][:, d, :],
                        start=(d == 0),
                        stop=(d == ND - 1),
                    )
                for d in range(ND):
                    nc.tensor.matmul(
                        pv[:, :cw],
                        lhsT=wgvs(hk, ND * P + d * P, P),
                        rhs=xc[ci][:, d, :],
                        start=(d == 0),
                        stop=(d == ND - 1),
                    )
                sg = act_pool.tile([P, 512], FP, tag="sg")
                nc.scalar.activation(sg[:, :cw], pg[:, :cw], AF.Silu)
                nc.vector.tensor_tensor(
                    ht[hk][:, cs:cs + cw], pv[:, :cw], sg[:, :cw], op=OP.mult
                )


    # ---- aux phase A: the quarter-shard's hT over NHA h-tiles -----------
    with ExitStack() as axctx:
        ps_a = axctx.enter_context(tc.tile_pool(name="psa", bufs=2, space="PSUM"))
        for k in range(NHA):
            pga = ps_a.tile([P, cap_a], FP, tag="pga", name=f"pga{k}")
            pva = ps_a.tile([P, cap_a], FP, tag="pva", name=f"pva{k}")
            for d in range(ND):
                nc.tensor.matmul(
                    pga[:],
                    lhsT=wgva[:, (2 * k) * ND * P + d * P:(2 * k) * ND * P + (d + 1) * P],
                    rhs=xa[:, d, :],
                    start=(d == 0),
                    stop=(d == ND - 1),
                )
            for d in range(ND):
                nc.tensor.matmul(
                    pva[:],
                    lhsT=wgva[:, (2 * k + 1) * ND * P + d * P:(2 * k + 1) * ND * P + (d + 1) * P],
                    rhs=xa[:, d, :],
                    start=(d == 0),
                    stop=(d == ND - 1),
                )
            sga = act_pool.tile([P, 512], FP, tag="sg")
            nc.scalar.activation(sga[:, :cap_a], pga[:], AF.Silu)
            nc.vector.tensor_tensor(hta[:, k, :], pva[:], sga[:, :cap_a], op=OP.mult)

    # ---- phase B: yT[d, tok] = (wo^T @ hT) * w --------------------------
    with ExitStack() as bctx:
        ps_y = bctx.enter_context(tc.tile_pool(name="psy", bufs=2, space="PSUM"))
        ps_ya = bctx.enter_context(tc.tile_pool(name="psya", bufs=1, space="PSUM"))
        # aux phase B first: all ND d-tiles packed column-wise into PSUM
        # groups, one DVE mult + one early store per group
        gsz = max(1, 512 // cap_a)          # d-tiles per PSUM tile
        ysa = yst_pool.tile([P, ND * cap_a], BF, tag="ya", name="ysa")
        for g0 in range(0, ND, gsz):
            gn = min(gsz, ND - g0)
            pya = ps_ya.tile([P, 512], FP, tag="auxy", name=f"pya{g0}")
            for j in range(gn):
                for k in range(NHA):
                    nc.tensor.matmul(
                        pya[:, j * cap_a:(j + 1) * cap_a],
                        lhsT=woa[:, k, (g0 + j) * P:(g0 + j + 1) * P],
                        rhs=hta[:, k, :],
                        start=(k == 0),
                        stop=(k == NHA - 1),
                    )
            nc.vector.tensor_tensor(
                ysa[:, g0 * cap_a:(g0 + gn) * cap_a],
                pya[:, :gn * cap_a],
                wbca[:, g0 * cap_a:(g0 + gn) * cap_a],
                op=OP.mult,
            )
        nc.scalar.dma_start(out=ya_d.ap(), in_=ysa[:])
        for dt in range(ND):
            ysb = yst_pool.tile([P, cap], BF, tag="y", name=f"y{dt}")
            if dt < ND - 1:
                pys = [
                    ps_y.tile([P, 512], FP, tag=f"c{ci}", name=f"py{dt}c{ci}")
                    for ci in range(len(chunks))
                ]
                # interleave chunks per hk: one wo weight-load feeds all
                # three chunk matmuls
                for hk in range(NH):
                    for ci, (cs, cw) in enumerate(chunks):
                        nc.tensor.matmul(
                            pys[ci][:, :cw],
                            lhsT=wo_t[:, hk, dt * P:(dt + 1) * P],
                            rhs=ht[hk][:, cs:cs + cw],
                            start=(hk == 0),
                            stop=(hk == NH - 1),
                        )
                for ci, (cs, cw) in enumerate(chunks):
                    nc.vector.tensor_tensor(
                        ysb[:, cs:cs + cw], pys[ci][:, :cw], wbc[:, cs:cs + cw],
                        op=OP.mult,
                    )
                nc.scalar.dma_start(
                    out=y_d.ap()[dt * P:(dt + 1) * P, :], in_=ysb[:]
                )
            else:
                # final d-tile: chunk-outer so earlier chunks scale + store
                # while later chunks are still accumulating, and the last
                # chunk is sub-split so the kernel's tail is one 68-column
                # store
                pieces = list(chunks)
                lcs, lcw = pieces[-1]
                if lcw > 96:
                    pieces[-1] = (lcs, lcw - 68)
                    pieces.append((lcs + lcw - 68, 68))
                pyp = [
                    ps_y.tile([P, 512], FP, tag=f"c{ci % len(chunks)}",
                              name=f"pyz{ci}")
                    for ci in range(len(pieces))
                ]
                for ci, (cs, cw) in enumerate(pieces):
                    for hk in range(NH):
                        nc.tensor.matmul(
                            pyp[ci][:, :cw],
                            lhsT=wo_t[:, hk, dt * P:(dt + 1) * P],
                            rhs=ht[hk][:, cs:cs + cw],
                            start=(hk == 0),
                            stop=(hk == NH - 1),
                        )
                    nc.vector.tensor_tensor(
                        ysb[:, cs:cs + cw], pyp[ci][:, :cw], wbc[:, cs:cs + cw],
                        op=OP.mult,
                    )
                    # scalar ring only: sync-ring stores have been observed
                    # to execute ~9us after their doorbell (late-queue
                    # pathology), gating the exec-ending NOTIFY
                    nc.scalar.dma_start(
                        out=y_d.ap()[dt * P:(dt + 1) * P, cs:cs + cw],
                        in_=ysb[:, cs:cs + cw],
                    )


def _build(cap, cap_a):
    nc = bacc.Bacc("TRN2", target_bir_lowering=False, debug=False)
    xt_d = nc.dram_tensor("xt", [P, ND * cap], BF, kind="ExternalInput")
    wgv_d = nc.dram_tensor("wgv", [P, NH * 2 * ND * P], BF, kind="ExternalInput")
    wo_d = nc.dram_tensor("wo", [H, D], BF, kind="ExternalInput")
    w_d = nc.dram_tensor("w", [1, cap], FP, kind="ExternalInput")
    y_d = nc.dram_tensor("y", [D, cap], BF, kind="ExternalOutput")
    xta_d = nc.dram_tensor("xta", [P, ND * cap_a], BF, kind="ExternalInput")
    wgva_d = nc.dram_tensor("wgva", [P, NHA * 2 * ND * P], BF, kind="ExternalInput")
    woa_d = nc.dram_tensor("woa", [NHA * P, D], BF, kind="ExternalInput")
    wa_d = nc.dram_tensor("wa", [1, ND * cap_a], FP, kind="ExternalInput")
    ya_d = nc.dram_tensor("ya", [P, ND * cap_a], BF, kind="ExternalOutput")
    with tile.TileContext(nc) as tc:
        with ExitStack() as ctx:
            _emit(nc, tc, ctx, cap, xt_d, wgv_d, wo_d, w_d, y_d,
                  cap_a, xta_d, wgva_d, woa_d, wa_d, ya_d)
    nc.compile()
    return nc


_NCS = {}


def _get_nc(cap, cap_a):
    if (cap, cap_a) not in _NCS:
        _NCS[(cap, cap_a)] = _build(cap, cap_a)
    return _NCS[(cap, cap_a)]


def _route(xf, gate_w, expert_bias):
    """Host router in float64: top-2 selection + normalized combine weights."""
    logits = xf.astype(np.float64) @ gate_w.astype(np.float64) + expert_bias.astype(
        np.float64
    )
    m = logits.max(axis=-1, keepdims=True)
    p = np.exp(logits - m)
    p /= p.sum(axis=-1, keepdims=True)
    # ties -> lower index first, matching jax.lax.top_k
    order = np.argsort(-p, axis=-1, kind="stable")[:, :TOP_K]
    rw = np.take_along_axis(p, order, axis=-1)
    rw = rw / (rw.sum(axis=-1, keepdims=True) + 1e-8)
    return order, rw


def _tile_wgv(wg, wv):
    """Two [D, H] fp32 -> [128, NH*2*ND*128] bf16, [p, hk, {g,v}, dt, h]."""
    def t(w):
        return w.astype(BF_NP).reshape(ND, P, NH, P).transpose(1, 2, 0, 3)

    return np.ascontiguousarray(
        np.stack([t(wg), t(wv)], axis=2).reshape(P, NH * 2 * ND * P)
    )


def kernel(x, gate_w, expert_bias, w_gate, w_value, w_out, _trace=False):
    x = np.asarray(x, dtype=np.float32)
    gate_w = np.asarray(gate_w, dtype=np.float32)
    expert_bias = np.asarray(expert_bias, dtype=np.float32)
    w_gate = np.asarray(w_gate, dtype=np.float32)
    w_value = np.asarray(w_value, dtype=np.float32)
    w_out = np.asarray(w_out, dtype=np.float32)

    xf = np.ascontiguousarray(x.reshape(N_TOK, D))
    order, rw = _route(xf, gate_w, expert_bias)
    idx = [np.flatnonzero((order == e).any(axis=-1)) for e in range(E)]
    loads = [len(i) for i in idx]
    max_load = max(1, max(loads))

    def r4(v):
        return max(4, -4 * (-v // 4))

    # main capacity: smallest (mult-of-4) cap with at most two overflow
    # experts; their overflow tokens are offloaded as H-quarter shards
    # spread over the 8 cores (4 shards per overflow expert).
    if max_load > CAP_LIMIT:
        cap, cap_a = CAP_LIMIT, 4      # multi-round fallback, no aux
    else:
        cap = r4(sorted(loads, reverse=True)[2])
        cap_a = r4(max([loads[e] - cap for e in range(E)] + [4]))
        while cap_a > 512:             # extreme imbalance: grow main cap
            cap = r4(cap + 128)
            cap_a = r4(max([loads[e] - cap for e in range(E)] + [4]))
    n_rounds = max(1, math.ceil(max_load / cap)) if max_load > CAP_LIMIT else 1
    ov = [e for e in range(E) if loads[e] > cap]     # <= 2 experts
    # shard s -> (expert, quarter); cores 0-3 serve ov[0], 4-7 serve ov[1]
    shards = [(ov[s // 4], s % 4) if s // 4 < len(ov) else None
              for s in range(E)]

    nc = _get_nc(cap, cap_a)
    wgv_t = [_tile_wgv(w_gate[e], w_value[e]) for e in range(E)]
    wo_t = [np.ascontiguousarray(w_out[e].astype(BF_NP)) for e in range(E)]
    # per-token combine weight of each token for expert e
    w_of = [
        np.where(
            order[:, 0] == e,
            rw[:, 0],
            np.where(order[:, 1] == e, rw[:, 1], 0.0),
        ).astype(np.float32)
        for e in range(E)
    ]

    def pack_x(ids, n, chunked):
        ids_p = np.zeros(n, dtype=np.int64)
        ids_p[: len(ids)] = ids
        xT = xf[ids_p].T.astype(BF_NP).reshape(ND, P, n)
        xT[:, :, len(ids):] = 0
        pieces = _chunks(n) if chunked else [(0, n)]
        return np.concatenate(
            [
                np.ascontiguousarray(
                    xT[:, :, cs:cs + cw].transpose(1, 0, 2)
                ).reshape(P, ND * cw)
                for cs, cw in pieces
            ],
            axis=1,
        )

    W2 = 2 * ND * P
    out = np.zeros((N_TOK, D), dtype=np.float32)
    last = None
    for r in range(n_rounds):
        in_maps = []
        for e in range(E):
            ids = idx[e][r * cap:(r + 1) * cap]
            w_pad = np.zeros((1, cap), dtype=np.float32)
            w_pad[0, : len(ids)] = w_of[e][ids]
            sh = shards[e] if n_rounds == 1 else None
            if sh is not None:
                ea, q = sh
                ids_a = idx[ea][cap:cap + cap_a]
                wgva = np.ascontiguousarray(
                    wgv_t[ea][:, q * NHA * W2:(q + 1) * NHA * W2])
                woa = np.ascontiguousarray(
                    wo_t[ea][q * NHA * P:(q + 1) * NHA * P, :])
                wa = np.zeros((1, cap_a), dtype=np.float32)
                wa[0, : len(ids_a)] = w_of[ea][ids_a]
            else:
                ids_a = np.zeros(0, dtype=np.int64)
                wgva = np.zeros((P, NHA * W2), dtype=BF_NP)
                woa = np.zeros((NHA * P, D), dtype=BF_NP)
                wa = np.zeros((1, cap_a), dtype=np.float32)
            in_maps.append({
                "xt": pack_x(ids, cap, True),
                "wgv": wgv_t[e],
                "wo": wo_t[e],
                "w": w_pad,
                "xta": pack_x(ids_a, cap_a, False),
                "wgva": wgva,
                "woa": woa,
                "wa": np.ascontiguousarray(np.tile(wa, ND)),
            })
        res = run_bass_kernel_spmd(
            nc, in_maps, core_ids=list(range(E)),
            trace=bool(_trace), trace_cores=list(range(E)) if _trace else None,
        )
        last = res
        for e in range(E):
            ids = idx[e][r * cap:(r + 1) * cap]
            if len(ids):
                out[ids] += res.results[e]["y"][:, : len(ids)].T.astype(np.float32)
            sh = shards[e] if n_rounds == 1 else None
            if sh is not None:
                ea, _ = sh
                ids_a = idx[ea][cap:cap + cap_a]
                if len(ids_a):
                    ya = res.results[e]["ya"].reshape(P, ND, cap_a)
                    contrib = ya.transpose(2, 1, 0).reshape(cap_a, D)
                    out[ids_a] += contrib[: len(ids_a)].astype(np.float32)
    if _trace:
        kernel.last_results = last
    return out.reshape(B, T, D)



# revision 32
# speedup vs baseline: 1.1850x; 1.1693x over previous
"""MoE FFN (8 experts, top-2, SwiGLU) Trainium2 kernel.

Expert-parallel with quarter-shard load balancing: core e holds expert
e's weights. The router (logits, softmax, top-2 selection AND combine
weights) is computed on the host in float64; tokens are dispatched to
the cores owning their top-2 experts, padded to a balanced main
capacity CAP = round4(3rd-largest expert load) so that at most two
experts overflow. Overflow tokens are offloaded as H-quarter shards:
every core additionally processes up to CAP_A tokens of one (expert,
H-quarter) pair (4 of the 16 h-tiles), so the two overflowing experts'
excess tokens are spread over all 8 cores at 1/4 cost each; the host
scatter-adds both the main y^T and the aux quarter-partials back into
token order. Each core runs the SwiGLU FFN entirely in bf16 (PE rate
matches fp32r while DMA/SBUF halve and LDWEIGHTS gets FWL), scaling by
the per-token combine weight.

Device structure:
  phase A (x-chunk outer):  hT[h, tok] = silu(x@wg)^T * (x@wv)^T
      stationary = wg/wv d-tiles [128, 128], moving = x^T token chunks.
  phase B (d-tile outer):   yT[d, tok] = wo^T @ hT, * w[tok]
      stationary = wo h-tiles [128, 128], moving = hT token chunks; the
      combine weight is applied as a DVE multiply against a
      partition-broadcast copy of w, and y^T stores are one DMA per
      d-tile.

DMA plan: two HWDGE rings drain concurrently. The sync ring carries, in
consumption order: the first x chunk and h-tile-0 weights (split into
halves for the earliest possible first matmul), the rest of the wg/wv
stream (one 512 KB DMA per h-tile), the w broadcast, the remaining x
chunks, wo, and finally the aux-shard weights/tokens — aux MUST ride
the tail of the sync ring: the tile scheduler hoists dependency-free
DMAs to the front of the stream, and only same-engine ring order can
keep the 3 MB aux burst behind the head loads that gate the first real
matmuls. The scalar ring carries only y stores, keeping store
descriptor-gen off the load-heavy sync sequencer. A burst of junk
matmuls on a memset tile ramps the HAM clock gate and bridges the DMA
ramp until the first loads land.

Self-contained: shapes hardcoded for x[2,2048,1024], 8 experts,
d_expert=2048, top-2; capacity adapts to the routed load at first call
(kernel compiled per distinct CAP and cached).
"""

import math
from contextlib import ExitStack

import ml_dtypes
import numpy as np

import concourse.bass as bass
import concourse.mybir as mybir
import concourse.tile as tile
from concourse import bacc
from concourse.bass_utils import run_bass_kernel_spmd

# ---- problem constants --------------------------------------------------
B, T, D = 2, 2048, 1024
N_TOK = B * T          # 4096 tokens
E = 8                  # experts == cores
H = 2048               # expert hidden dim
TOP_K = 2
P = 128
ND = D // P            # 8  d-tiles (contraction tiles of d_model)
NH = H // P            # 16 h-tiles
NWO = 4                # wo DMA blocks (4 h-tiles each)
NHA = NH // 4          # 4  h-tiles per aux quarter-shard

CAP_LIMIT = 1280       # SBUF budget bound; beyond this, dispatch in rounds
# 48 warm-up matmuls end ~12.5us, exactly when the ramping DMA can first
# sustain the chunk-0 sweep without stalls; starting real matmuls earlier
# (N_WARM=32, deps ready ~10.8us) was measured SLOWER — the stream then
# stalls on the wgv supply and the idle gaps cool the HAM clock gate.
N_WARM = 48

FP = mybir.dt.float32
BF = mybir.dt.bfloat16
AF = mybir.ActivationFunctionType
OP = mybir.AluOpType
BF_NP = ml_dtypes.bfloat16


def _chunks(cap):
    """Token-chunk schedule: a narrow first chunk so phase A can start on
    a small x load, then 512-wide chunks (the PSUM bank limit)."""
    first = min(320, cap)
    out = [(0, first)]
    s = first
    while s < cap:
        out.append((s, min(512, cap - s)))
        s += out[-1][1]
    return out


def _emit(nc, tc, ctx, cap, xt_d, wgv_d, wo_d, w_d, y_d,
          cap_a, xta_d, wgva_d, woa_d, wa_d, ya_d):
    chunks = _chunks(cap)
    const = ctx.enter_context(tc.tile_pool(name="const", bufs=1))
    x_pool = ctx.enter_context(tc.tile_pool(name="x", bufs=1))
    wgv_pool = ctx.enter_context(tc.tile_pool(name="wgv", bufs=1))
    wo_pool = ctx.enter_context(tc.tile_pool(name="wo", bufs=1))
    ht_pool = ctx.enter_context(tc.tile_pool(name="ht", bufs=1))
    act_pool = ctx.enter_context(tc.tile_pool(name="act", bufs=3))
    yst_pool = ctx.enter_context(tc.tile_pool(name="yst", bufs=2))

    # x arrives host-pre-tiled per chunk: xt_d[p, ND*cs + dt*cw + c], so
    # each chunk load is one contiguous 2*ND*cw-byte run per partition.
    xc = []
    for ci, (cs, cw) in enumerate(chunks):
        xtile = x_pool.tile([P, ND, cw], BF, tag=f"x{ci}", name=f"xc{ci}")
        xc.append(xtile)

    # sync ring, in consumption order: the first x chunk and wg/wv of
    # h-tile 0 land first (split into halves so the d0-3 matmuls of the
    # first PSUM group can start on a 0.5 MB footprint), then the rest
    # of the wg/wv stream, the remaining x chunks, then wo.
    # wgv tiles: per-h-tile for hk 0-7 (fine-grained completion gates the
    # chunk-0 sweep), two 4-h-tile blocks for hk 8-15 (consumed late;
    # fewer copies -> fewer queue descriptors to drain in the tail)
    W2 = 2 * ND * P
    cw0 = chunks[0][1]
    wgv_view = {}
    for hk in range(8):
        t = wgv_pool.tile([P, W2], BF, tag=f"wgv{hk}", name=f"wgv{hk}")
        wgv_view[hk] = (t, 0)
    for b in range(2):
        t = wgv_pool.tile([P, 4 * W2], BF, tag=f"wgvb{b}", name=f"wgvb{b}")
        for j in range(4):
            wgv_view[8 + b * 4 + j] = (t, j * W2)

    def wgvs(hk, off, size):
        t, base = wgv_view[hk]
        return t[:, base + off:base + off + size]

    # x0 and wgv0 interleave on the sync ring (moving x0 to the scalar
    # ring was tried and made the deps LAND LATER — the scalar HWDGE
    # queue group spins up worse for head loads)
    x0_ap = xt_d.ap()[:, 0:ND * cw0].rearrange("p (dt c) -> p dt c", dt=ND)
    nc.sync.dma_start(out=xc[0][:, 0:ND // 2, :], in_=x0_ap[:, 0:ND // 2, :])
    nc.sync.dma_start(out=wgvs(0, 0, ND * P), in_=wgv_d.ap()[:, :ND * P])
    nc.sync.dma_start(out=xc[0][:, ND // 2:, :], in_=x0_ap[:, ND // 2:, :])
    nc.sync.dma_start(out=wgvs(0, ND * P, ND * P), in_=wgv_d.ap()[:, ND * P:W2])
    for hk in range(1, 8):
        nc.sync.dma_start(
            out=wgvs(hk, 0, W2), in_=wgv_d.ap()[:, hk * W2:(hk + 1) * W2]
        )
    for b in range(2):
        hk0 = 8 + b * 4
        nc.sync.dma_start(
            out=wgv_view[hk0][0][:],
            in_=wgv_d.ap()[:, hk0 * W2:(hk0 + 4) * W2],
        )

    # w broadcast rides the sync ring behind the head loads — its DRE
    # replicate-write packets would otherwise steal SDMA slots from the
    # critical first x/weight loads; it is only needed at phase B.
    wbc = const.tile([P, cap], FP)
    nc.sync.dma_start(out=wbc[:], in_=w_d.ap().partition_broadcast(P))
    for ci, (cs, cw) in enumerate(chunks[1:], start=1):
        nc.sync.dma_start(
            out=xc[ci][:],
            in_=xt_d.ap()[:, ND * cs:ND * (cs + cw)]
            .rearrange("p (dt c) -> p dt c", dt=ND),
        )
    wo_t = wo_pool.tile([P, NH, D], BF, tag="wo", name="wo")
    nc.sync.dma_start(
        out=wo_t[:], in_=wo_d.ap().rearrange("(j p) c -> p j c", j=NH)
    )

    # aux shard tiles (loads issued after the warmup block: their payload
    # is only consumed after main phase A, and issuing the descriptors
    # first would delay the scalar engine's Silu table preload)
    W2A = NHA * 2 * ND * P
    wgva = wgv_pool.tile([P, W2A], BF, tag="wgva", name="wgva")
    woa = wo_pool.tile([P, NHA, D], BF, tag="woa", name="woa")
    xa = x_pool.tile([P, ND, cap_a], BF, tag="xa", name="xa")
    wbca = const.tile([P, ND * cap_a], FP)
    hta = ht_pool.tile([P, NHA, cap_a], BF, tag="hta", name="hta")

    ht = [
        ht_pool.tile([P, cap], BF, tag=f"h{hk}", name=f"ht{hk}") for hk in range(NH)
    ]

    # ---- PE warm-up: ramp the HAM clock gate while the first loads land
    ones = const.tile([P, P], BF)
    nc.vector.memset(ones[:], 1.0)
    # preload the ACT engine's Silu table during the DMA wait — the first
    # real activation would otherwise pay the ~1.3us table load and stall
    # the first PSUM drain mid-sweep
    sg_warm = act_pool.tile([P, 512], FP, tag="sg", name="sg_warm")
    nc.scalar.activation(sg_warm[:, :1], ones[:, :1], AF.Silu)
    with ExitStack() as wctx:
        ps_w = wctx.enter_context(tc.tile_pool(name="psw", bufs=1, space="PSUM"))
        warm = ps_w.tile([E, P], FP, name="warm", tag="warm")
        for _ in range(N_WARM):
            nc.tensor.matmul(
                warm[:], lhsT=ones[:, :E], rhs=ones[:], start=True, stop=True
            )

    # aux loads ride the SYNC ring behind the wo loads: same-engine DMA
    # program order is preserved through the ring, so the 3 MB aux burst
    # cannot be scheduler-hoisted ahead of the critical head loads (on the
    # scalar ring the scheduler floated it to the very front, stalling the
    # first real matmuls ~5us behind the wgva transfer)
    nc.sync.dma_start(out=wgva[:], in_=wgva_d.ap())
    nc.sync.dma_start(
        out=woa[:], in_=woa_d.ap().rearrange("(j p) c -> p j c", j=NHA)
    )
    nc.sync.dma_start(
        out=xa[:], in_=xta_d.ap().rearrange("p (dt c) -> p dt c", dt=ND)
    )
    nc.sync.dma_start(out=wbca[:], in_=wa_d.ap().partition_broadcast(P))

    # ---- phase A: hT[h, tok] = silu(x@wg)^T * (x@wv)^T ------------------
    with ExitStack() as actx:
        ps_g = actx.enter_context(tc.tile_pool(name="psg", bufs=2, space="PSUM"))
        ps_v = actx.enter_context(tc.tile_pool(name="psv", bufs=2, space="PSUM"))
        # the final chunk sweep uses its own two banks so phase B's PSUM
        # pool reuses banks freed a whole sweep earlier, not A's last drains
        ps_g2 = actx.enter_context(tc.tile_pool(name="psg2", bufs=1, space="PSUM"))
        ps_v2 = actx.enter_context(tc.tile_pool(name="psv2", bufs=1, space="PSUM"))
        for ci, (cs, cw) in enumerate(chunks):
            last_ci = ci == len(chunks) - 1
            for hk in range(NH):
                pg = (ps_g2 if last_ci else ps_g).tile(
                    [P, 512], FP, tag="pg", name=f"pg{ci}_{hk}"
                )
                pv = (ps_v2 if last_ci else ps_v).tile(
                    [P, 512], FP, tag="pv", name=f"pv{ci}_{hk}"
                )
                for d in range(ND):
                    nc.tensor.matmul(
                        pg[:, :cw],
                        lhsT=wgvs(hk, d * P, P),
                        rhs=xc[ci][:, d, :],
                        start=(d == 0),
                        stop=(d == ND - 1),
                    )
                for d in range(ND):
                    nc.tensor.matmul(
                        pv[:, :cw],
                        lhsT=wgvs(hk, ND * P + d * P, P),
                        rhs=xc[ci][:, d, :],
                        start=(d == 0),
                        stop=(d == ND - 1),
                    )
                sg = act_pool.tile([P, 512], FP, tag="sg")
                nc.scalar.activation(sg[:, :cw], pg[:, :cw], AF.Silu)
                nc.vector.tensor_tensor(
                    ht[hk][:, cs:cs + cw], pv[:, :cw], sg[:, :cw], op=OP.mult
                )


    # ---- aux phase A: the quarter-shard's hT over NHA h-tiles -----------
    with ExitStack() as axctx:
        ps_a = axctx.enter_context(tc.tile_pool(name="psa", bufs=2, space="PSUM"))
        for k in range(NHA):
            pga = ps_a.tile([P, cap_a], FP, tag="pga", name=f"pga{k}")
            pva = ps_a.tile([P, cap_a], FP, tag="pva", name=f"pva{k}")
            for d in range(ND):
                nc.tensor.matmul(
                    pga[:],
                    lhsT=wgva[:, (2 * k) * ND * P + d * P:(2 * k) * ND * P + (d + 1) * P],
                    rhs=xa[:, d, :],
                    start=(d == 0),
                    stop=(d == ND - 1),
                )
            for d in range(ND):
                nc.tensor.matmul(
                    pva[:],
                    lhsT=wgva[:, (2 * k + 1) * ND * P + d * P:(2 * k + 1) * ND * P + (d + 1) * P],
                    rhs=xa[:, d, :],
                    start=(d == 0),
                    stop=(d == ND - 1),
                )
            sga = act_pool.tile([P, 512], FP, tag="sg")
            nc.scalar.activation(sga[:, :cap_a], pga[:], AF.Silu)
            nc.vector.tensor_tensor(hta[:, k, :], pva[:], sga[:, :cap_a], op=OP.mult)

    # ---- phase B: yT[d, tok] = (wo^T @ hT) * w --------------------------
    with ExitStack() as bctx:
        ps_y = bctx.enter_context(tc.tile_pool(name="psy", bufs=2, space="PSUM"))
        ps_ya = bctx.enter_context(tc.tile_pool(name="psya", bufs=1, space="PSUM"))
        # aux phase B first: all ND d-tiles packed column-wise into PSUM
        # groups, one DVE mult + one early store per group
        gsz = max(1, 512 // cap_a)          # d-tiles per PSUM tile
        ysa = yst_pool.tile([P, ND * cap_a], BF, tag="ya", name="ysa")
        for g0 in range(0, ND, gsz):
            gn = min(gsz, ND - g0)
            pya = ps_ya.tile([P, 512], FP, tag="auxy", name=f"pya{g0}")
            for j in range(gn):
                for k in range(NHA):
                    nc.tensor.matmul(
                        pya[:, j * cap_a:(j + 1) * cap_a],
                        lhsT=woa[:, k, (g0 + j) * P:(g0 + j + 1) * P],
                        rhs=hta[:, k, :],
                        start=(k == 0),
                        stop=(k == NHA - 1),
                    )
            nc.vector.tensor_tensor(
                ysa[:, g0 * cap_a:(g0 + gn) * cap_a],
                pya[:, :gn * cap_a],
                wbca[:, g0 * cap_a:(g0 + gn) * cap_a],
                op=OP.mult,
            )
        nc.scalar.dma_start(out=ya_d.ap(), in_=ysa[:])
        for dt in range(ND):
            ysb = yst_pool.tile([P, cap], BF, tag="y", name=f"y{dt}")
            if dt < ND - 1:
                pys = [
                    ps_y.tile([P, 512], FP, tag=f"c{ci}", name=f"py{dt}c{ci}")
                    for ci in range(len(chunks))
                ]
                # interleave chunks per hk: one wo weight-load feeds all
                # three chunk matmuls
                for hk in range(NH):
                    for ci, (cs, cw) in enumerate(chunks):
                        nc.tensor.matmul(
                            pys[ci][:, :cw],
                            lhsT=wo_t[:, hk, dt * P:(dt + 1) * P],
                            rhs=ht[hk][:, cs:cs + cw],
                            start=(hk == 0),
                            stop=(hk == NH - 1),
                        )
                for ci, (cs, cw) in enumerate(chunks):
                    nc.vector.tensor_tensor(
                        ysb[:, cs:cs + cw], pys[ci][:, :cw], wbc[:, cs:cs + cw],
                        op=OP.mult,
                    )
                nc.scalar.dma_start(
                    out=y_d.ap()[dt * P:(dt + 1) * P, :], in_=ysb[:]
                )
            else:
                # final d-tile: chunk-outer so earlier chunks scale + store
                # while later chunks are still accumulating, and the last
                # chunk is sub-split so the kernel's tail is one 68-column
                # store
                pieces = list(chunks)
                lcs, lcw = pieces[-1]
                if lcw > 96:
                    pieces[-1] = (lcs, lcw - 68)
                    pieces.append((lcs + lcw - 68, 68))
                pyp = [
                    ps_y.tile([P, 512], FP, tag=f"c{ci % len(chunks)}",
                              name=f"pyz{ci}")
                    for ci in range(len(pieces))
                ]
                for ci, (cs, cw) in enumerate(pieces):
                    for hk in range(NH):
                        nc.tensor.matmul(
                            pyp[ci][:, :cw],
                            lhsT=wo_t[:, hk, dt * P:(dt + 1) * P],
                            rhs=ht[hk][:, cs:cs + cw],
                            start=(hk == 0),
                            stop=(hk == NH - 1),
                        )
                    nc.vector.tensor_tensor(
                        ysb[:, cs:cs + cw], pyp[ci][:, :cw], wbc[:, cs:cs + cw],
                        op=OP.mult,
                    )
                    # scalar ring only: sync-ring stores have been observed
                    # to execute ~9us after their doorbell (late-queue
                    # pathology), gating the exec-ending NOTIFY
                    nc.scalar.dma_start(
                        out=y_d.ap()[dt * P:(dt + 1) * P, cs:cs + cw],
                        in_=ysb[:, cs:cs + cw],
                    )


def _build(cap, cap_a):
    nc = bacc.Bacc("TRN2", target_bir_lowering=False, debug=False)
    xt_d = nc.dram_tensor("xt", [P, ND * cap], BF, kind="ExternalInput")
    wgv_d = nc.dram_tensor("wgv", [P, NH * 2 * ND * P], BF, kind="ExternalInput")
    wo_d = nc.dram_tensor("wo", [H, D], BF, kind="ExternalInput")
    w_d = nc.dram_tensor("w", [1, cap], FP, kind="ExternalInput")
    y_d = nc.dram_tensor("y", [D, cap], BF, kind="ExternalOutput")
    xta_d = nc.dram_tensor("xta", [P, ND * cap_a], BF, kind="ExternalInput")
    wgva_d = nc.dram_tensor("wgva", [P, NHA * 2 * ND * P], BF, kind="ExternalInput")
    woa_d = nc.dram_tensor("woa", [NHA * P, D], BF, kind="ExternalInput")
    wa_d = nc.dram_tensor("wa", [1, ND * cap_a], FP, kind="ExternalInput")
    ya_d = nc.dram_tensor("ya", [P, ND * cap_a], BF, kind="ExternalOutput")
    with tile.TileContext(nc) as tc:
        with ExitStack() as ctx:
            _emit(nc, tc, ctx, cap, xt_d, wgv_d, wo_d, w_d, y_d,
                  cap_a, xta_d, wgva_d, woa_d, wa_d, ya_d)
    nc.compile()
    return nc


_NCS = {}


def _get_nc(cap, cap_a):
    if (cap, cap_a) not in _NCS:
        _NCS[(cap, cap_a)] = _build(cap, cap_a)
    return _NCS[(cap, cap_a)]


def _route(xf, gate_w, expert_bias):
    """Host router in float64: top-2 selection + normalized combine weights."""
    logits = xf.astype(np.float64) @ gate_w.astype(np.float64) + expert_bias.astype(
        np.float64
    )
    m = logits.max(axis=-1, keepdims=True)
    p = np.exp(logits - m)
    p /= p.sum(axis=-1, keepdims=True)
    # ties -> lower index first, matching jax.lax.top_k
    order = np.argsort(-p, axis=-1, kind="stable")[:, :TOP_K]
    rw = np.take_along_axis(p, order, axis=-1)
    rw = rw / (rw.sum(axis=-1, keepdims=True) + 1e-8)
    return order, rw


def _tile_wgv(wg, wv):
    """Two [D, H] fp32 -> [128, NH*2*ND*128] bf16, [p, hk, {g,v}, dt, h]."""
    def t(w):
        return w.astype(BF_NP).reshape(ND, P, NH, P).transpose(1, 2, 0, 3)

    return np.ascontiguousarray(
        np.stack([t(wg), t(wv)], axis=2).reshape(P, NH * 2 * ND * P)
    )


def kernel(x, gate_w, expert_bias, w_gate, w_value, w_out, _trace=False):
    x = np.asarray(x, dtype=np.float32)
    gate_w = np.asarray(gate_w, dtype=np.float32)
    expert_bias = np.asarray(expert_bias, dtype=np.float32)
    w_gate = np.asarray(w_gate, dtype=np.float32)
    w_value = np.asarray(w_value, dtype=np.float32)
    w_out = np.asarray(w_out, dtype=np.float32)

    xf = np.ascontiguousarray(x.reshape(N_TOK, D))
    order, rw = _route(xf, gate_w, expert_bias)
    idx = [np.flatnonzero((order == e).any(axis=-1)) for e in range(E)]
    loads = [len(i) for i in idx]
    max_load = max(1, max(loads))

    def r4(v):
        return max(4, -4 * (-v // 4))

    # main capacity: smallest (mult-of-4) cap with at most two overflow
    # experts; their overflow tokens are offloaded as H-quarter shards
    # spread over the 8 cores (4 shards per overflow expert).
    if max_load > CAP_LIMIT:
        cap, cap_a = CAP_LIMIT, 4      # multi-round fallback, no aux
    else:
        cap = r4(sorted(loads, reverse=True)[2])
        cap_a = r4(max([loads[e] - cap for e in range(E)] + [4]))
        while cap_a > 512:             # extreme imbalance: grow main cap
            cap = r4(cap + 128)
            cap_a = r4(max([loads[e] - cap for e in range(E)] + [4]))
    n_rounds = max(1, math.ceil(max_load / cap)) if max_load > CAP_LIMIT else 1
    ov = [e for e in range(E) if loads[e] > cap]     # <= 2 experts
    # shard s -> (expert, quarter); cores 0-3 serve ov[0], 4-7 serve ov[1]
    shards = [(ov[s // 4], s % 4) if s // 4 < len(ov) else None
              for s in range(E)]

    nc = _get_nc(cap, cap_a)
    wgv_t = [_tile_wgv(w_gate[e], w_value[e]) for e in range(E)]
    wo_t = [np.ascontiguousarray(w_out[e].astype(BF_NP)) for e in range(E)]
    # per-token combine weight of each token for expert e
    w_of = [
        np.where(
            order[:, 0] == e,
            rw[:, 0],
            np.where(order[:, 1] == e, rw[:, 1], 0.0),
        ).astype(np.float32)
        for e in range(E)
    ]

    def pack_x(ids, n, chunked):
        ids_p = np.zeros(n, dtype=np.int64)
        ids_p[: len(ids)] = ids
        xT = xf[ids_p].T.astype(BF_NP).reshape(ND, P, n)
        xT[:, :, len(ids):] = 0
        pieces = _chunks(n) if chunked else [(0, n)]
        return np.concatenate(
            [
                np.ascontiguousarray(
                    xT[:, :, cs:cs + cw].transpose(1, 0, 2)
                ).reshape(P, ND * cw)
                for cs, cw in pieces
            ],
            axis=1,
        )

    W2 = 2 * ND * P
    out = np.zeros((N_TOK, D), dtype=np.float32)
    last = None
    for r in range(n_rounds):
        in_maps = []
        for e in range(E):
            ids = idx[e][r * cap:(r + 1) * cap]
            w_pad = np.zeros((1, cap), dtype=np.float32)
            w_pad[0, : len(ids)] = w_of[e][ids]
            sh = shards[e] if n_rounds == 1 else None
            if sh is not None:
                ea, q = sh
                ids_a = idx[ea][cap:cap + cap_a]
                wgva = np.ascontiguousarray(
                    wgv_t[ea][:, q * NHA * W2:(q + 1) * NHA * W2])
                woa = np.ascontiguousarray(
                    wo_t[ea][q * NHA * P:(q + 1) * NHA * P, :])
                wa = np.zeros((1, cap_a), dtype=np.float32)
                wa[0, : len(ids_a)] = w_of[ea][ids_a]
            else:
                ids_a = np.zeros(0, dtype=np.int64)
                wgva = np.zeros((P, NHA * W2), dtype=BF_NP)
                woa = np.zeros((NHA * P, D), dtype=BF_NP)
                wa = np.zeros((1, cap_a), dtype=np.float32)
            in_maps.append({
                "xt": pack_x(ids, cap, True),
                "wgv": wgv_t[e],
                "wo": wo_t[e],
                "w": w_pad,
                "xta": pack_x(ids_a, cap_a, False),
                "wgva": wgva,
                "woa": woa,
                "wa": np.ascontiguousarray(np.tile(wa, ND)),
            })
        res = run_bass_kernel_spmd(
            nc, in_maps, core_ids=list(range(E)),
            trace=bool(_trace), trace_cores=list(range(E)) if _trace else None,
        )
        last = res
        for e in range(E):
            ids = idx[e][r * cap:(r + 1) * cap]
            if len(ids):
                out[ids] += res.results[e]["y"][:, : len(ids)].T.astype(np.float32)
            sh = shards[e] if n_rounds == 1 else None
            if sh is not None:
                ea, _ = sh
                ids_a = idx[ea][cap:cap + cap_a]
                if len(ids_a):
                    ya = res.results[e]["ya"].reshape(P, ND, cap_a)
                    contrib = ya.transpose(2, 1, 0).reshape(cap_a, D)
                    out[ids_a] += contrib[: len(ids_a)].astype(np.float32)
    if _trace:
        kernel.last_results = last
    return out.reshape(B, T, D)

